# revision 1
# baseline (speedup 1.0000x reference)
"""Trainium2 Bass kernel for nn_Attention_58652073394851.

out[n] = sum_s alpha_s[n] * Z_s[n],  alpha_s = softmax_N(tanh(Z_s @ W_s.T + b_s.T) @ q)

Strategy (8 NeuronCores, data-parallel over N):
  - Host shards N=100000 into 8 chunks of 12500, zero-pads each to 12544 rows
    (98 tiles of 128), and passes Z.T per stream (host-transposed) so the
    score matmul can stream Z with D on partitions, plus Z natural for
    streams C/F for the output pass.
  - Stream T's transposed copy stays RESIDENT in SBUF (98KB/partition); the
    output pass recovers its natural-layout tiles with PE transposes, so
    Z_T crosses HBM once instead of twice.
  - Phase 1 (per core): h.T = tanh(W Z.T + b) via PE matmuls (K=128 x2
    halves), score columns via per-tile matmul h @ q -> scores [128, 98].
  - exp(s) without max-subtraction (|s| <= ||q||_1 ~ 6.5, no overflow in
    f32); row sums via ACT accum_out; partition sum via PE matmul with ones;
    host-computed padding-row contribution subtracted.
  - One AllGather of the 3 per-stream local sums (12B per core), summed
    on-chip via a K=8 matmul.
  - Phase 2: out_tile = sum_s alpha_s[:,t] * Z_s[t]  (ACT mul reading the
    transposed-back PSUM tile + 2 DVE scalar_tensor_tensor fused mul-adds).
"""

import os as _os

import numpy as np

N_TOTAL = 100000
D = 256
H = 64
NCORES = 8
PN = N_TOTAL // NCORES          # 12500 real rows per core
TILES = 98                      # padded tiles of 128 rows
ROWS = TILES * 128              # 12544 padded rows per core

# chunking: phase 1/2 process 8 tiles (1024 rows) per DMA
_CT = int(_os.environ.get("K_CHUNK", "7"))
CHUNKS = [_CT] * (TILES // _CT) + ([TILES % _CT] if TILES % _CT else [])

# float32r streams f32 through the PE at ~4x the f32 rate but rounds the
# mantissa (measured 2.1e-4 rel err vs 2.7e-6 for f32). Off by default.
USE_F32R = _os.environ.get("K_F32R", "0") == "1"
# keep Z_T's transposed copy resident in SBUF; phase 2 transposes it back
# on the PE instead of re-reading Z_T from HBM.
RESIDENT_T = _os.environ.get("K_RESIDENT", "1") == "1"

_CACHE = {}


def _build_program(collective=True):
    import concourse.bacc as bacc
    import concourse.mybir as mybir
    from concourse import masks
    from concourse.tile import TileContext
    from contextlib import ExitStack

    f32 = mybir.dt.float32
    AF = mybir.ActivationFunctionType
    ALU = mybir.AluOpType
    zdt = mybir.dt.float32r if USE_F32R else f32

    nc = bacc.Bacc(None, target_bir_lowering=False, num_devices=NCORES)

    zt_d = [nc.dram_tensor(f"zt_{s}", [D, ROWS], zdt, kind="ExternalInput")
            for s in "TCF"]
    nat_streams = [1, 2] if RESIDENT_T else [0, 1, 2]
    zn_d = {s: nc.dram_tensor(f"zn_{'TCF'[s]}", [ROWS, D], f32,
                              kind="ExternalInput")
            for s in nat_streams}
    wt_d = nc.dram_tensor("wt", [128, 2, 3, H], zdt, kind="ExternalInput")
    bq_d = nc.dram_tensor("bq", [H, 4], f32, kind="ExternalInput")
    # per-stream sum of exp(score) over this core's PAD rows (host-computed:
    # pad rows have Z=0 -> score = tanh(b_s) . q, identical for all pads)
    padc_d = nc.dram_tensor("padc", [1, 3], f32, kind="ExternalInput")
    out_d = nc.dram_tensor("out", [ROWS, D], f32, kind="ExternalOutput")

    zn_v = {s: z.rearrange("(t p) d -> p t d", p=128) for s, z in zn_d.items()}
    out_v = out_d.rearrange("(t p) d -> p t d", p=128)

    with TileContext(nc) as tc, ExitStack() as ctx:
        const = ctx.enter_context(tc.tile_pool(name="const", bufs=1))
        persist = ctx.enter_context(tc.tile_pool(name="persist", bufs=1))
        io1b = int(_os.environ.get("K_IO1B", "3"))
        io2b = int(_os.environ.get("K_IO2B", "4"))
        io1 = ctx.enter_context(tc.tile_pool(name="io1", bufs=io1b))
        w1b = int(_os.environ.get("K_W1B", "4"))
        work1 = ctx.enter_context(tc.tile_pool(name="work1", bufs=w1b))
        io2 = ctx.enter_context(tc.tile_pool(name="io2", bufs=io2b))
        w2b = int(_os.environ.get("K_W2B", "2"))
        work2 = ctx.enter_context(tc.tile_pool(name="work2", bufs=w2b))
        ps_h = ctx.enter_context(tc.tile_pool(name="ps_h", bufs=2, space="PSUM"))
        ps_s = ctx.enter_context(tc.tile_pool(name="ps_s", bufs=2, space="PSUM"))
        ps_t = ctx.enter_context(tc.tile_pool(name="ps_t", bufs=2, space="PSUM"))
        ps_m = ctx.enter_context(tc.tile_pool(name="ps_m", bufs=2, space="PSUM"))
        dram = ctx.enter_context(tc.tile_pool(name="dram", bufs=1, space="DRAM"))

        wt_sb = const.tile([128, 2, 3, H], zdt)
        nc.sync.dma_start(wt_sb[:], wt_d[:])
        bq_sb = const.tile([H, 4], f32)
        nc.sync.dma_start(bq_sb[:], bq_d[:])
        padc_sb = const.tile([1, 3], f32)
        nc.sync.dma_start(padc_sb[:], padc_d[:])
        ones_col = const.tile([128, 1], f32)
        nc.vector.memset(ones_col[:], 1.0)
        ones_row = const.tile([1, 128], f32)
        nc.vector.memset(ones_row[:], 1.0)
        zero128 = const.tile([128, 1], f32)
        nc.vector.memset(zero128[:], 0.0)
        if RESIDENT_T:
            ident = const.tile([128, 128], f32)
            masks.make_identity(nc, ident[:])
            ztres = persist.tile([128, 2, ROWS], zdt, tag="ztres")

        score = [persist.tile([128, TILES], f32, tag=f"score{s}",
                              name=f"score{s}")
                 for s in range(3)]
        alpha = [persist.tile([128, TILES], f32, tag=f"alpha{s}",
                              name=f"alpha{s}")
                 for s in range(3)]
        rowsum = persist.tile([128, 3], f32, tag="rowsum")

        # ---------------- phase 1: scores ----------------
        for s in range(3):
            t0 = 0
            for ct in CHUNKS:
                ncols = ct * 128
                c_lo = t0 * 128
                if RESIDENT_T and s == 0:
                    zt0 = ztres[:, 0, :]
                    zt1 = ztres[:, 1, :]
                    o0 = c_lo
                else:
                    zt0t = io1.tile([128, _CT * 128], zdt, tag="zt0")
                    zt1t = io1.tile([128, _CT * 128], zdt, tag="zt1")
                    zt0, zt1 = zt0t[:, :], zt1t[:, :]
                    o0 = 0
                nc.sync.dma_start(zt0[:, o0:o0 + ncols],
                                  zt_d[s][0:128, c_lo:c_lo + ncols])
                nc.sync.dma_start(zt1[:, o0:o0 + ncols],
                                  zt_d[s][128:256, c_lo:c_lo + ncols])
                sp = ps_s.tile([128, _CT], f32, tag="sp")
                for g0 in range(0, ct, 4):
                    gt = min(4, ct - g0)
                    gc = gt * 128
                    c0 = o0 + g0 * 128
                    hp = ps_h.tile([H, 512], f32, tag="hp")
                    nc.tensor.matmul(hp[:, 0:gc], wt_sb[:, 0, s, :],
                                     zt0[:, c0:c0 + gc], start=True, stop=False)
                    nc.tensor.matmul(hp[:, 0:gc], wt_sb[:, 1, s, :],
                                     zt1[:, c0:c0 + gc], start=False, stop=True)
                    ht = work1.tile([H, 512], f32, tag="ht")
                    nc.scalar.activation(ht[:, 0:gc], hp[:, 0:gc], AF.Tanh,
                                         bias=bq_sb[:, s:s + 1])
                    for j in range(gt):
                        nc.tensor.matmul(sp[:, g0 + j:g0 + j + 1],
                                         ht[:, j * 128:(j + 1) * 128],
                                         bq_sb[:, 3:4])
                nc.vector.tensor_copy(score[s][:, t0:t0 + ct], sp[:, 0:ct])
                t0 += ct

        # exp + per-partition row sums
        for s in range(3):
            nc.scalar.activation(alpha[s][:], score[s][:], AF.Exp,
                                 bias=zero128[:], accum_out=rowsum[:, s:s + 1])

        # local sums [1,3] via PE partition-reduce, minus the padding rows'
        # contribution (so they don't enter the softmax denominator)
        sl_ps = ps_m.tile([1, 3], f32, tag="m", name="sl_ps")
        nc.tensor.matmul(sl_ps[:], ones_col[:], rowsum[:])
        sl_sb = persist.tile([1, 3], f32, tag="slsb")
        nc.vector.tensor_tensor(sl_sb[:], sl_ps[:], padc_sb[:],
                                op=ALU.subtract)

        # ---------------- AllGather + on-chip sum ----------------
        sg_sb = persist.tile([1, 3], f32, tag="sgsb")
        if collective:
            cc_in = dram.tile([1, 3], f32, tag="ccin")
            cc_out = dram.tile([NCORES, 3], f32, tag="ccout")
            nc.gpsimd.dma_start(cc_in[:], sl_sb[:])
            nc.gpsimd.collective_compute(
                "AllGather", ALU.bypass,
                replica_groups=[list(range(NCORES))],
                ins=[cc_in[:].opt()],
                outs=[cc_out[:].opt()],
            )
            ag_sb = persist.tile([NCORES, 3], f32, tag="agsb")
            nc.gpsimd.dma_start(ag_sb[:], cc_out[:])
            sg_ps = ps_m.tile([1, 3], f32, tag="m", name="sg_ps")
            nc.tensor.matmul(sg_ps[:], ones_col[0:NCORES, :], ag_sb[:])
            nc.vector.tensor_copy(sg_sb[:], sg_ps[:])
        else:
            # single-core timeline-sim variant: pretend local sum is global
            nc.vector.tensor_copy(sg_sb[:], sl_sb[:])

        inv_sb = persist.tile([1, 3], f32, tag="invsb")
        nc.vector.reciprocal(inv_sb[:], sg_sb[:])
        bc_ps = ps_m.tile([128, 3], f32, tag="m", name="bc_ps")
        nc.tensor.matmul(bc_ps[:], ones_row[:], inv_sb[:])
        invb = persist.tile([128, 3], f32, tag="invb")
        nc.vector.tensor_copy(invb[:], bc_ps[:])

        # alpha = exp(s) / S_global   (in place)
        for s in range(3):
            nc.vector.tensor_scalar_mul(alpha[s][:], alpha[s][:],
                                        invb[:, s:s + 1])

        # ---------------- phase 2: weighted sum ----------------
        t0 = 0
        for ct in CHUNKS:
            zn = {}
            for s in nat_streams:
                znt = io2.tile([128, _CT, D], f32, tag=f"zn{s}", name=f"zn{s}")
                nc.sync.dma_start(znt[:, 0:ct, :], zn_v[s][:, t0:t0 + ct, :])
                zn[s] = znt
            ob = work2.tile([128, _CT, D], f32, tag="ob")
            for j in range(ct):
                t = t0 + j
                c0 = t * 128
                if RESIDENT_T:
                    tp = ps_t.tile([128, D], f32, tag="tp")
                    zt0 = ztres[:, 0, c0:c0 + 128]
                    zt1 = ztres[:, 1, c0:c0 + 128]
                    if USE_F32R:
                        zt0, zt1 = zt0.bitcast(f32), zt1.bitcast(f32)
                    nc.tensor.transpose(tp[:, 0:128], zt0, ident[:])
                    nc.tensor.transpose(tp[:, 128:256], zt1, ident[:])
                    src_t = tp[:, :]
                else:
                    src_t = zn[0][:, j, :]
                nc.scalar.activation(ob[:, j, :], src_t, AF.Copy,
                                     scale=alpha[0][:, t:t + 1])
                nc.vector.scalar_tensor_tensor(
                    ob[:, j, :], zn[1][:, j, :], alpha[1][:, t:t + 1],
                    ob[:, j, :], op0=ALU.mult, op1=ALU.add)
                nc.vector.scalar_tensor_tensor(
                    ob[:, j, :], zn[2][:, j, :], alpha[2][:, t:t + 1],
                    ob[:, j, :], op0=ALU.mult, op1=ALU.add)
            out_eng = (nc.scalar if _os.environ.get("K_OUTDMA", "sync") == "scalar"
                       else nc.sync)
            out_eng.dma_start(out_v[:, t0:t0 + ct, :], ob[:, 0:ct, :])
            t0 += ct

    nc.compile()
    return nc


def _get_program():
    if "nc" not in _CACHE:
        _CACHE["nc"] = _build_program()
    return _CACHE["nc"]


def _prep_in_maps(inputs):
    f32 = np.float32
    Zs = [np.ascontiguousarray(np.asarray(inputs[f"Z_{s}"], dtype=f32))
          for s in "TCF"]
    Ws = [np.asarray(inputs[f"W_{s}"], dtype=f32) for s in "TCF"]
    bs = [np.asarray(inputs[f"b_{s}"], dtype=f32) for s in "TCF"]
    q = np.asarray(inputs["q"], dtype=f32)

    # wt_pack[p, h, s, j] = W_s[j, h*128 + p]
    wt = np.stack([W.T.reshape(2, 128, H) for W in Ws])       # [3, 2, 128, 64]
    wt_pack = np.ascontiguousarray(wt.transpose(2, 1, 0, 3))  # [128, 2, 3, 64]
    bq = np.ascontiguousarray(np.concatenate(bs + [q], axis=1))  # [64, 4]
    # padding rows have Z=0 -> score = tanh(b_s).q; their exp contribution
    # is removed from the local softmax denominator on-device
    padc = np.array([[(ROWS - PN) * np.exp(np.tanh(b[:, 0]) @ q[:, 0])
                      for b in bs]], dtype=f32)

    in_maps = []
    for i in range(NCORES):
        m = {"wt": wt_pack, "bq": bq, "padc": padc}
        for s, name in enumerate("TCF"):
            zp = np.zeros((ROWS, D), dtype=f32)
            zp[:PN] = Zs[s][i * PN:(i + 1) * PN]
            if not (RESIDENT_T and s == 0):
                m[f"zn_{name}"] = zp
            m[f"zt_{name}"] = np.ascontiguousarray(zp.T)
        in_maps.append(m)
    return in_maps


LAST_RESULTS = None


def kernel(**inputs) -> np.ndarray:
    global LAST_RESULTS
    from concourse.bass_utils import run_bass_kernel_spmd

    nc = _get_program()
    in_maps = _prep_in_maps(inputs)
    res = run_bass_kernel_spmd(nc, in_maps, core_ids=list(range(NCORES)))
    LAST_RESULTS = res
    out = np.concatenate([res.results[i]["out"][:PN] for i in range(NCORES)],
                         axis=0)
    return out


if __name__ == "__main__":
    rng = np.random.default_rng(0)
    ins = {
        "Z_T": rng.standard_normal((N_TOTAL, D), dtype=np.float32),
        "Z_C": rng.standard_normal((N_TOTAL, D), dtype=np.float32),
        "Z_F": rng.standard_normal((N_TOTAL, D), dtype=np.float32),
        "W_T": rng.standard_normal((H, D), dtype=np.float32) / 8,
        "b_T": rng.standard_normal((H, 1), dtype=np.float32) / 8,
        "W_C": rng.standard_normal((H, D), dtype=np.float32) / 8,
        "b_C": rng.standard_normal((H, 1), dtype=np.float32) / 8,
        "W_F": rng.standard_normal((H, D), dtype=np.float32) / 8,
        "b_F": rng.standard_normal((H, 1), dtype=np.float32) / 8,
        "q": rng.standard_normal((H, 1), dtype=np.float32) / 8,
    }
    out = kernel(**ins)
    print(out.shape, out.dtype)

